# revision 1
# baseline (speedup 1.0000x reference)
"""AttentionSink masked-add kernel for 8 TRN2 NeuronCores.

out[b,h,i,j] = w[b,h,i,j] + mask[i,j], mask 0 where allowed else -1e30.
Allowed: j < 4 (sink) or i-25 <= j <= i (local band).

Since |w| << ulp(-1e30) in fp32, masked outputs are exactly -1e30. The
kernel therefore:
  1. writes the whole output with the constant -1e30 from a small SBUF tile
     (stride-0 broadcast DMA source, full 8 KiB rows, ~peak HBM write BW);
  2. overwrites the allowed positions by copying them straight from the
     input: the sink columns as a thin DRAM->DRAM copy, and the local band
     as a DRAM->DRAM copy over a diagonal access pattern (stride S+1), which
     covers exactly the 26-wide allowed parallelogram for row blocks r>=1 —
     no arithmetic needed since the mask is 0 there.
Only block r=0 (rows 0..127, where the band clips at column 0) goes through
SBUF with a real mask add. Total HBM traffic per core: ~134 MB written +
~11 MB read, ~1.5% of the input read.

The 64 (S,S) matrices are split 8 per core; no collectives.
"""

import sys

import numpy as np

try:
    import concourse.bass as bass
except ImportError:  # fresh environment: add the repo staging paths
    for p in ("/opt/trn_rl_repo", "/root/.axon_site/_ro/trn_rl_repo"):
        if p not in sys.path:
            sys.path.append(p)
    import concourse.bass as bass

import concourse.tile as tile
from concourse import bacc, mybir
from concourse.bass_utils import run_bass_kernel_spmd

B, H, S = 4, 16, 2048
SINK = 4
LEFT = 25
NEG = -1e30
P = 128                    # SBUF partitions / rows per block
NBLK = S // P              # 16 row blocks per matrix
N_CORES = 8
M = (B * H) // N_CORES     # matrices per core


def _host_masks():
    # mask for rows 0..127 x cols 0..127 (sink + clamped band; block 0)
    i = np.arange(P)[:, None]
    j = np.arange(P)[None, :]
    allowed0 = (j < SINK) | ((j >= i - LEFT) & (j <= i))
    return np.where(allowed0, 0.0, NEG).astype(np.float32)


def _build_program(repeat=1):
    nc = bacc.Bacc(
        "TRN2", target_bir_lowering=False, debug=False, num_devices=N_CORES
    )
    dt = mybir.dt.float32
    x = nc.dram_tensor("x", [M, S, S], dt, kind="ExternalInput").ap()
    mf = nc.dram_tensor("mask_first", [P, P], dt, kind="ExternalInput").ap()
    out = nc.dram_tensor("out", [M, S, S], dt, kind="ExternalOutput").ap()

    def bcast_m(ap2d, m=M):
        # (p, w) SBUF AP -> (p, m, w) with stride-0 middle dim
        (ps, pn), (ws, wn) = ap2d.ap
        return bass.AP(ap2d.tensor, ap2d.offset, [[ps, pn], [0, m], [ws, wn]])

    with tile.TileContext(nc) as tc:
        with tc.tile_pool(name="pool", bufs=1) as pool:
            # constant -1e30 background row, split memset across two engines
            c = pool.tile([P, S], dt, name="c")
            nc.vector.memset(c[:, 0 : S * 5 // 9], NEG)
            nc.gpsimd.memset(c[:, S * 5 // 9 : S], NEG)

            # block-0 mask and band data
            mf_t = pool.tile([P, P], dt, name="mf_t")
            nc.gpsimd.dma_start(mf_t[:], mf[:])
            bt0 = pool.tile([P, M, P], dt, name="bt0")
            nc.gpsimd.dma_start(
                bt0[:], x[:, 0:P, 0:P].rearrange("m p w -> p m w")
            )
            nc.vector.tensor_add(bt0[:], bt0[:], bcast_m(mf_t[:]))

            for _rep in range(repeat):
              # Block 0 is emitted LAST: the band copy below overlaps the
              # r>=1 const stores but not block 0's, so its post-wait
              # descriptor prep hides behind block 0's transfer.
              for r in list(range(1, NBLK)) + [0]:
                R = r * P
                # constant background store (near-8 KiB contiguous rows);
                # alternates the two HWDGE rings so transfers pipeline
                # back-to-back. Starts at col 4 (rows >= 128) / col 128
                # (block 0) so the sink / block-0 stores below overlap no
                # const store and can dispatch without waiting.
                lo = P if r == 0 else SINK
                ceng = nc.sync if r % 2 == 0 else nc.scalar
                ceng.dma_start(
                    out[:, R : R + P, lo:S].rearrange("m p c -> p m c"),
                    bcast_m(c[:, lo:S]),
                )

              # rows 0..127, cols 0..127: computed sink+clamped-band block via
              # SBUF (SWDGE; no dependency on any const store)
              nc.gpsimd.dma_start(
                  out[:, 0:P, 0:P].rearrange("m p w -> p m w"), bt0[:]
              )
              # sink columns rows 128..2047: one thin DRAM->DRAM passthrough
              nc.sync.dma_start(
                  out[:, P:S, 0:SINK], x[:, P:S, 0:SINK]
              )
              # band rows 128..2047: one DRAM->DRAM copy over the diagonal
              # parallelograms: out[m, r*128+p, r*128-25+p+q], q in [0, 26)
              off = P * S + (P - LEFT)
              dims = [
                  [S * S, M],
                  [P * (S + 1), NBLK - 1],
                  [S + 1, P],
                  [1, LEFT + 1],
              ]
              nc.scalar.dma_start(
                  bass.AP(out.tensor, off, dims),
                  bass.AP(x.tensor, off, dims),
              )

    nc.compile()
    return nc


_CACHE = {}


def _get_nc():
    if "nc" not in _CACHE:
        _CACHE["nc"] = _build_program()
    return _CACHE["nc"]


def _in_maps(w):
    mask_first = _host_masks()
    flat = w.reshape(B * H, S, S)
    return [
        {"x": flat[i * M : (i + 1) * M], "mask_first": mask_first}
        for i in range(N_CORES)
    ]


def _gather(chunks):
    """Stack per-core (M,S,S) results along axis 0. Zero-copy when they are
    consecutive contiguous slices of one base buffer (bass2jax returns views
    of a single concatenated array); otherwise fall back to a copy."""
    try:
        c0 = chunks[0]
        step = c0.nbytes
        ptr0 = c0.__array_interface__["data"][0]
        base = c0.base
        if base is not None and all(
            c.base is base
            and c.flags["C_CONTIGUOUS"]
            and c.__array_interface__["data"][0] == ptr0 + i * step
            for i, c in enumerate(chunks)
        ):
            # one shared owner + adjacent layout: a strided view over c0
            # (whose .base keeps the owner alive) covers all of them
            return np.lib.stride_tricks.as_strided(
                c0,
                shape=(len(chunks),) + c0.shape,
                strides=(step,) + c0.strides,
            )
    except Exception:
        pass
    return np.concatenate([c[None] for c in chunks], axis=0)


def kernel(attention_weights, seq_len=None):
    w = np.ascontiguousarray(np.asarray(attention_weights, dtype=np.float32))
    assert w.shape == (B, H, S, S)
    nc = _get_nc()
    in_maps = _in_maps(w)
    res = run_bass_kernel_spmd(nc, in_maps, core_ids=list(range(N_CORES)))
    out = _gather([res.results[i]["out"] for i in range(N_CORES)])
    return out.reshape(B, H, S, S)



# revision 18
# speedup vs baseline: 1.0104x; 1.0104x over previous
"""AttentionSink masked-add kernel for 8 TRN2 NeuronCores.

out[b,h,i,j] = w[b,h,i,j] + mask[i,j], mask 0 where allowed else -1e30.
Allowed: j < 4 (sink) or i-25 <= j <= i (local band).

Since |w| << ulp(-1e30) in fp32, masked outputs are exactly -1e30. The
kernel writes the constant background with wrap-around diagonal chunks:
for each row i >= 127, the masked span [row i, cols i+1..2047] ++
[row i+1, cols 0..i-25] is one contiguous 2023-element run in flat DRAM
(stride S+1 between consecutive runs). One dma_start per 128-row block
covers the whole inter-band constant region with 8092-byte descriptors
and ZERO overlap with the band, so the allowed band is written exactly
once (thin diagonal DRAM->DRAM passthrough) instead of const+overwrite.
Only the 4-wide sink columns are double-written (const then overwritten
by a thin passthrough copy, 0.25 MB).

Block 0 (rows 0..127): cols 0..127 go through SBUF with a real mask add
(the band clips at col 0 there); cols 128..2047 of rows 0..126 are one
rectangular const store (row 127's tail is covered by the first wrap
chunk).

Per-core HBM traffic: ~134.8 MB written + ~2.9 MB read; the only excess
over the output size is the 0.25 MB sink double-write.

The 64 (S,S) matrices are split 8 per core; no collectives.
"""

import sys

import numpy as np

try:
    import concourse.bass as bass
except ImportError:  # fresh environment: add the repo staging paths
    for p in ("/opt/trn_rl_repo", "/root/.axon_site/_ro/trn_rl_repo"):
        if p not in sys.path:
            sys.path.append(p)
    import concourse.bass as bass

import concourse.tile as tile
from concourse import bacc, mybir
from concourse.bass_utils import run_bass_kernel_spmd

B, H, S = 4, 16, 2048
SINK = 4
LEFT = 25
NEG = -1e30
P = 128                    # SBUF partitions / rows per block
NBLK = S // P              # 16 row blocks per matrix
N_CORES = 8
M = (B * H) // N_CORES     # matrices per core
CLEN = S - LEFT            # 2023: wrap-around const chunk length
W0 = 208                   # width of the computed block-0 store


def _build_program():
    nc = bacc.Bacc(
        "TRN2", target_bir_lowering=False, debug=False, num_devices=N_CORES
    )
    dt = mybir.dt.float32
    x = nc.dram_tensor("x", [M, S, S], dt, kind="ExternalInput").ap()
    out = nc.dram_tensor("out", [M, S, S], dt, kind="ExternalOutput").ap()

    def bcast_m(ap2d, m=M):
        # (p, w) SBUF AP -> (p, m, w) with stride-0 middle dim
        (ps, pn), (ws, wn) = ap2d.ap
        return bass.AP(ap2d.tensor, ap2d.offset, [[ps, pn], [0, m], [ws, wn]])

    with tile.TileContext(nc) as tc:
        with tc.tile_pool(name="pool", bufs=1) as pool:
            # block-0 band data + mask loads go first, on the two HWDGE
            # queues so their dispatch pipelines overlap. bt0 is widened to
            # W0 cols (cols P..W0 memset to NEG, not loaded) so its store is
            # big enough to hide the sink copy's sem-propagation chain.
            bt0 = pool.tile([P, M, W0], dt, name="bt0")
            nc.sync.dma_start(
                bt0[:, :, 0:P], x[:, 0:P, 0:P].rearrange("m p w -> p m w")
            )

            # constant -1e30 background row, memset split across two engines
            # (balanced for their elem/ns rates so both finish together)
            c = pool.tile([P, CLEN], dt, name="c")
            nc.vector.memset(c[:, 0:934], NEG)
            nc.gpsimd.memset(c[:, 934:CLEN], NEG)
            nc.gpsimd.memset(bt0[:, :, P:W0], NEG)

            # block-0 mask applied in place via two affine selects over
            # cols 4..127 (cols 0..3 are the always-allowed sink):
            # keep x where j <= p, then where j >= p - 25; else -1e30.
            # iota(p, m, jj) = base + p*cm + pattern steps, j = 4 + jj.
            nc.gpsimd.affine_select(
                bt0[:, :, SINK:P],
                bt0[:, :, SINK:P],
                [[0, M], [-1, P - SINK]],
                mybir.AluOpType.is_ge,
                NEG,
                base=-SINK,
                channel_multiplier=1,
            )
            nc.gpsimd.affine_select(
                bt0[:, :, SINK:P],
                bt0[:, :, SINK:P],
                [[0, M], [1, P - SINK]],
                mybir.AluOpType.is_ge,
                NEG,
                base=SINK + LEFT,
                channel_multiplier=-1,
            )

            # Wrap-around diagonal const chunks: chunk i (i = 127..2046)
            # covers [row i, cols i+1..2047] ++ [row i+1, cols 0..i-25],
            # one contiguous 2023-elem run at flat offset i*(S+1)+1.
            # Emitted per 128-chunk block, alternating the two HWDGE rings.
            for r in range(1, NBLK):
                i0 = r * P - 1
                off = i0 * (S + 1) + 1
                dims = [[S + 1, P], [S * S, M], [1, CLEN]]
                src = bass.AP(
                    c.tensor, c[:].offset, [[c[:].ap[0][0], P], [0, M], [1, CLEN]]
                )
                ceng = nc.sync if r % 2 == 1 else nc.scalar
                ceng.dma_start(bass.AP(out.tensor, off, dims), src)

            # rows 0..111, cols W0..2047: rectangular const store (row 127's
            # right tail is covered by wrap chunk i=127 above; cols 128..W0
            # come from the widened bt0 store)
            nc.scalar.dma_start(
                out[:, 0:112, W0:S].rearrange("m p c -> p m c"),
                bcast_m(c[0:112, 0 : S - W0]),
            )

            # band rows 128..2047: one DRAM->DRAM copy over the diagonal
            # parallelograms: out[m, r*128+p, r*128-25+p+q], q in [0, 26).
            # Disjoint from the wrap chunks: written exactly once.
            boff = P * S + (P - LEFT)
            bdims = [
                [S * S, M],
                [P * (S + 1), NBLK - 1],
                [S + 1, P],
                [1, LEFT + 1],
            ]
            nc.scalar.dma_start(
                bass.AP(out.tensor, boff, bdims),
                bass.AP(x.tensor, boff, bdims),
            )

            # fillers while the sink copy's wait on the band resolves: the
            # blk0-const remainder and the widened computed block-0 store.
            # (Tile's list scheduler hoists one no-dep DMA ahead of the
            # waiting band copy; each piece alone still covers the chain.)
            nc.scalar.dma_start(
                out[:, 112 : P - 1, W0:S].rearrange("m p c -> p m c"),
                bcast_m(c[112 : P - 1, 0 : S - W0]),
            )
            nc.scalar.dma_start(
                out[:, 0:P, 0:W0].rearrange("m p w -> p m w"), bt0[:]
            )

            # sink columns rows 128..2047: thin DRAM->DRAM passthrough,
            # overwrites the -1e30 the wrap chunks put at cols 0..3
            nc.scalar.dma_start(
                out[:, P:S, 0:SINK], x[:, P:S, 0:SINK]
            )

    nc.compile()
    return nc


_CACHE = {}


def _get_nc():
    if "nc" not in _CACHE:
        _CACHE["nc"] = _build_program()
    return _CACHE["nc"]


def _in_maps(w):
    flat = w.reshape(B * H, S, S)
    return [
        {"x": flat[i * M : (i + 1) * M]} for i in range(N_CORES)
    ]


def _gather(chunks):
    """Stack per-core (M,S,S) results along axis 0. Zero-copy when they are
    consecutive contiguous slices of one base buffer (bass2jax returns views
    of a single concatenated array); otherwise fall back to a copy."""
    try:
        c0 = chunks[0]
        step = c0.nbytes
        ptr0 = c0.__array_interface__["data"][0]
        base = c0.base
        if base is not None and all(
            c.base is base
            and c.flags["C_CONTIGUOUS"]
            and c.__array_interface__["data"][0] == ptr0 + i * step
            for i, c in enumerate(chunks)
        ):
            # one shared owner + adjacent layout: a strided view over c0
            # (whose .base keeps the owner alive) covers all of them
            return np.lib.stride_tricks.as_strided(
                c0,
                shape=(len(chunks),) + c0.shape,
                strides=(step,) + c0.strides,
            )
    except Exception:
        pass
    return np.concatenate([c[None] for c in chunks], axis=0)


def kernel(attention_weights, seq_len=None):
    w = np.ascontiguousarray(np.asarray(attention_weights, dtype=np.float32))
    assert w.shape == (B, H, S, S)
    nc = _get_nc()
    in_maps = _in_maps(w)
    res = run_bass_kernel_spmd(nc, in_maps, core_ids=list(range(N_CORES)))
    out = _gather([res.results[i]["out"] for i in range(N_CORES)])
    return out.reshape(B, H, S, S)
